# revision 6
# baseline (speedup 1.0000x reference)
"""CP-decomposed conv (1x1 -> depthwise-h -> depthwise-w -> 1x1) on 8 TRN2
NeuronCores, data-parallel over batch (4 images per core).

MIXED fold design (v1 was all-folded, PE-bound at 227us busy / 249us total):
  - Per band, the h-conv is either FOLDED into the stage-A matmul
    (w1 = f1 (x) f3, 3x PE columns, v1 style; band height 8) or UNFOLDED
    (A uses w3 = f3 only; h-conv as ratio taps on DVE; band height 6 so
    both band types use 8 u-rows = one [128,1024] PSUM tile).
    The folded:unfolded row split is the PE<->vector-engine balance knob.
  - Taps use tensor_scalar (4x mode) + tensor_tensor (2x) instead of STT.
    GPSIMD (idle in v1) takes tensor_tensor adds for a subset of bands.
  - OUTPUT INT8 with analytic per-filter scales (s_f = 4.3*sigma_f/127,
    sigma_f exact from the CP factors): same engine cost as v1's bf16
    PSUM->SBUF copies but halves output DMA. Host dequantizes.
    Adds ~1e-2 rel err -> total ~1.1e-2 vs the 2e-2 gate.
  - PSUM: ups [128,1024] x2 bufs + ops [128,1008] x2 bufs = 8 banks.
    Stage-B PSUM tiles are allocated per-ft from the pool ring (a single
    shared tile raced: ft0's quant read overlapped ft1's matmuls).
"""

import numpy as np

B, C, H, W = 32, 256, 128, 128
FH, FW = 3, 3
F, R = 256, 128
HP, WP = H - FH + 1, W - FW + 1  # 126, 126
NCORES = 8
BL = B // NCORES  # images per core

QCLIP = 4.3  # int8 clip at QCLIP * sigma_f

# Band plan: k folded bands (bh=8) + (n-k) unfolded bands (bh=6),
# 8k + 6(n-k) = 126  =>  k = 63 - 3n. n=18 -> 9+9, n=19 -> 6+13.
N_BANDS = 18


def _bands():
    n = N_BANDS
    k = 63 - 3 * n
    assert 0 <= k <= n
    kinds = []
    nf, nu = k, n - k
    # interleave folded/unfolded as evenly as possible
    a, b = (True, False) if nf >= nu else (False, True)
    hi, lo = max(nf, nu), min(nf, nu)
    ratio = hi / max(lo, 1)
    ca = cb = 0
    for i in range(n):
        if cb * ratio <= ca and cb < lo:
            kinds.append(b)
            cb += 1
        else:
            kinds.append(a)
            ca += 1
    out = []
    h0 = 0
    for i, f in enumerate(kinds):
        bh = 8 if f else 6
        out.append((i, h0, bh, f))
        h0 += bh
    assert h0 == HP
    return out


BANDS = _bands()

# --- engine-assignment knobs (by band index) ---
GP_W = set(range(0, N_BANDS, 2))       # bands whose w-adds run on GPSIMD
ZE_ACT = set(range(N_BANDS))           # bands whose ze copy runs on ACT
Q_DVE = set(range(3, N_BANDS, 4))      # bands whose quant runs on DVE

_NC_CACHE = {}


def _build_nc():
    import concourse.bacc as bacc
    import concourse.mybir as mybir
    import concourse.tile as tile

    dt = mybir.dt
    bf16 = dt.bfloat16
    f32 = dt.float32
    i8 = dt.int8
    mult = mybir.AluOpType.mult
    add = mybir.AluOpType.add

    nc = bacc.Bacc("TRN2", target_bir_lowering=False, debug=False,
                   num_devices=NCORES)

    x_d = nc.dram_tensor("x", [BL, C, H, W], bf16, kind="ExternalInput").ap()
    w1_d = nc.dram_tensor("w1", [FH * C, R], bf16, kind="ExternalInput").ap()
    w3_d = nc.dram_tensor("w3", [C, R], bf16, kind="ExternalInput").ap()
    # f0s: [R, 2, F]: variant 0 for folded bands (f0 * f2[0]),
    #                 variant 1 for unfolded (f0 * f1[0] * f2[0])
    f0s_d = nc.dram_tensor("f0s", [R, 2, F], bf16, kind="ExternalInput").ap()
    taps_d = nc.dram_tensor("taps", [R, 4], f32, kind="ExternalInput").ap()
    qs_d = nc.dram_tensor("qs", [R, 2], f32, kind="ExternalInput").ap()
    out_d = nc.dram_tensor("out", [BL, 2, 128, HP * WP], i8,
                           kind="ExternalOutput").ap()

    with tile.TileContext(nc, trace_sim=False) as tc:
        with tc.tile_pool(name="wp", bufs=1) as wp, \
             tc.tile_pool(name="xp", bufs=2) as xp, \
             tc.tile_pool(name="zep", bufs=2) as zep, \
             tc.tile_pool(name="t12", bufs=2) as t12p, \
             tc.tile_pool(name="zhp", bufs=2) as zhp, \
             tc.tile_pool(name="zp", bufs=2) as zp, \
             tc.tile_pool(name="op", bufs=2) as op, \
             tc.tile_pool(name="ups", bufs=2, space="PSUM") as upsp, \
             tc.tile_pool(name="ops", bufs=4, space="PSUM") as opsp:

            # --- weights (resident) ---
            w1_t = wp.tile([128, FH * 2, 128], bf16)  # [c_sub, a*2+ct, r]
            nc.scalar.dma_start(
                w1_t[:, :, :], w1_d.rearrange("(kt p) r -> p kt r", p=128))
            w3_t = wp.tile([128, 2, 128], bf16)
            nc.scalar.dma_start(
                w3_t[:, :, :], w3_d.rearrange("(ct p) r -> p ct r", p=128))
            f0s_t = wp.tile([128, 2, F], bf16)
            nc.scalar.dma_start(f0s_t[:, :, :], f0s_d)
            taps_t = wp.tile([128, 4], f32)
            nc.scalar.dma_start(taps_t[:, :], taps_d)
            qs_t = wp.tile([128, 2], f32)
            nc.scalar.dma_start(qs_t[:, :], qs_d)

            for img in range(BL):
                # --- load x image: [c_sub, ct, h*w] ---
                x_t = xp.tile([128, 2, H * W], bf16, tag="x")
                qparts = ([(0, 8), (8, 8)] if img == 0 else [(0, 16)]) \
                    + [(16 * q, 16) for q in range(1, 8)]
                for (row0, nrow) in qparts:
                    for ct in range(2):
                        nc.sync.dma_start(
                            x_t[:, ct, row0 * 128:(row0 + nrow) * 128],
                            x_d[img, ct * 128:(ct + 1) * 128,
                                row0:row0 + nrow, :],
                        )

                for (bi, h0, bh, folded) in BANDS:
                    nzh = bh * 128
                    nz = bh * WP
                    off = h0 * 128
                    u_ps = upsp.tile([128, 8 * 128], f32, tag="u")

                    if folded:
                        # --- A folded: u = zh directly (3a x 2ct) ---
                        for a in range(FH):
                            for ct in range(2):
                                for c0 in range(0, nzh, 512):
                                    n = min(512, nzh - c0)
                                    nc.tensor.matmul(
                                        u_ps[:, c0:c0 + n],
                                        w1_t[:, a * 2 + ct, :],
                                        x_t[:, ct,
                                            off + a * 128 + c0:
                                            off + a * 128 + c0 + n],
                                        start=(a == 0 and ct == 0),
                                        stop=(a == FH - 1 and ct == 1),
                                    )
                        nu = nzh
                    else:
                        # --- A unfolded: u = y rows h0..h0+bh+2 ---
                        nu = (bh + 2) * 128
                        for ct in range(2):
                            for c0 in range(0, nu, 512):
                                n = min(512, nu - c0)
                                nc.tensor.matmul(
                                    u_ps[:, c0:c0 + n],
                                    w3_t[:, ct, :],
                                    x_t[:, ct, off + c0: off + c0 + n],
                                    start=(ct == 0), stop=(ct == 1),
                                )

                    # --- ze: PSUM -> SBUF bf16 (unscaled) ---
                    ze_t = zep.tile([128, 8 * 128], bf16, tag="ze")
                    if bi in ZE_ACT:
                        nc.scalar.copy(ze_t[:, 0:nu], u_ps[:, 0:nu])
                    else:
                        nc.vector.tensor_copy(ze_t[:, 0:nu], u_ps[:, 0:nu])

                    if folded:
                        zh_t = ze_t  # already h-convolved
                    else:
                        # --- h-taps (DVE): zh = ze + rh1*ze[+1] + rh2*ze[+2]
                        t1_t = t12p.tile([128, 6 * 128], bf16, tag="t1")
                        t2_t = t12p.tile([128, 6 * 128], bf16, tag="t2")
                        zh_t = zhp.tile([128, 8 * 128], bf16, tag="zh")
                        nc.vector.tensor_scalar(
                            t1_t[:, 0:nzh], ze_t[:, 128:128 + nzh],
                            taps_t[:, 0:1], None, mult)
                        nc.vector.tensor_scalar(
                            t2_t[:, 0:nzh], ze_t[:, 256:256 + nzh],
                            taps_t[:, 1:2], None, mult)
                        nc.vector.tensor_tensor(
                            zh_t[:, 0:nzh], ze_t[:, 0:nzh],
                            t1_t[:, 0:nzh], add)
                        nc.vector.tensor_tensor(
                            zh_t[:, 0:nzh], zh_t[:, 0:nzh],
                            t2_t[:, 0:nzh], add)

                    # --- w-taps: z = zh + rw1*zh[+1] + rw2*zh[+2] ---
                    z_t = zp.tile([128, 8 * WP], bf16, tag="z")
                    zh3 = zh_t[:, 0:nzh].rearrange("p (h w) -> p h w", w=128)
                    z3 = z_t[:, 0:nz].rearrange("p (h w) -> p h w", w=WP)
                    t3 = t12p.tile([128, 8 * WP], bf16, tag="t3")
                    t4 = t12p.tile([128, 8 * WP], bf16, tag="t4")
                    t33 = t3[:, 0:nz].rearrange("p (h w) -> p h w", w=WP)
                    t43 = t4[:, 0:nz].rearrange("p (h w) -> p h w", w=WP)
                    nc.vector.tensor_scalar(
                        t33, zh3[:, :, 1:1 + WP], taps_t[:, 2:3], None, mult)
                    nc.vector.tensor_scalar(
                        t43, zh3[:, :, 2:2 + WP], taps_t[:, 3:4], None, mult)
                    if bi in GP_W:
                        nc.gpsimd.tensor_tensor(
                            z3, zh3[:, :, 0:WP], t33, add)
                        nc.gpsimd.tensor_tensor(z3, z3, t43, add)
                    else:
                        nc.vector.tensor_tensor(
                            z3, zh3[:, :, 0:WP], t33, add)
                        nc.vector.tensor_tensor(z3, z3, t43, add)

                    # --- stage B + int8 quant (v1 idiom: per-chunk psum)
                    fv = 0 if folded else 1
                    o_t = op.tile([128, 2, 8 * WP], i8, tag="o")
                    for ft in range(2):
                        for c0 in range(0, nz, 504):
                            n = min(504, nz - c0)
                            o_ps = opsp.tile([128, 504], f32, tag="ops")
                            nc.tensor.matmul(
                                o_ps[:, 0:n],
                                f0s_t[:, fv, ft * 128:(ft + 1) * 128],
                                z_t[:, c0:c0 + n],
                                start=True, stop=True,
                            )
                            if bi in Q_DVE:
                                nc.vector.tensor_scalar(
                                    o_t[:, ft, c0:c0 + n], o_ps[:, 0:n],
                                    qs_t[:, ft:ft + 1], None, mult)
                            else:
                                nc.scalar.mul(o_t[:, ft, c0:c0 + n],
                                              o_ps[:, 0:n],
                                              qs_t[:, ft:ft + 1])
                    nc.sync.dma_start(
                        out_d[img, :, :, h0 * WP:(h0 + bh) * WP].rearrange(
                            "f p n -> p f n"),
                        o_t[:, :, 0:nz],
                    )

    nc.compile()
    return nc


def _get_nc():
    if "nc" not in _NC_CACHE:
        _NC_CACHE["nc"] = _build_nc()
    return _NC_CACHE["nc"]


def _prep(x, f0, f1, f2, f3):
    import ml_dtypes
    bf16 = ml_dtypes.bfloat16

    f0 = np.asarray(f0, np.float64)
    f1 = np.asarray(f1, np.float64)
    f2 = np.asarray(f2, np.float64)
    f3 = np.asarray(f3, np.float64)

    h0 = f1[0].copy()
    h0[np.abs(h0) < 1e-30] = 1e-30
    w0 = f2[0].copy()
    w0[np.abs(w0) < 1e-30] = 1e-30
    taps = np.stack([f1[1] / h0, f1[2] / h0, f2[1] / w0, f2[2] / w0],
                    axis=1).astype(np.float32)

    # w1[(a, c), r] = f1[a, r] * f3[c, r]  (folded-band stage A)
    w1 = (f1[:, None, :] * f3[None, :, :]).reshape(FH * C, R)
    w1b = np.ascontiguousarray(w1.astype(np.float32).astype(bf16))
    w3b = np.ascontiguousarray(f3.astype(np.float32).astype(bf16))

    # stage-B weights: [R, 2, F]; variant 0 folded (x f2[0]),
    # variant 1 unfolded (x f1[0]*f2[0])
    f0s = np.stack([(f0 * w0[None, :]).T,
                    (f0 * (h0 * w0)[None, :]).T], axis=1)
    f0sb = np.ascontiguousarray(f0s.astype(np.float32).astype(bf16))

    # analytic per-filter sigma: K[f,c,a,w] = sum_r f0 f1 f2 f3
    K = np.einsum('fr,ar,wr,cr->fcaw', f0, f1, f2, f3, optimize=True)
    sig = np.sqrt((K ** 2).sum(axis=(1, 2, 3)))
    s_f = (QCLIP * sig / 127.0).astype(np.float32)  # [F]
    qs = np.ascontiguousarray(
        (1.0 / s_f).reshape(2, 128).T.astype(np.float32))  # [128, 2]

    xb = np.ascontiguousarray(np.asarray(x).astype(bf16))
    in_maps = [
        {"x": xb[i * BL:(i + 1) * BL], "w1": w1b, "w3": w3b, "f0s": f0sb,
         "taps": np.ascontiguousarray(taps), "qs": qs}
        for i in range(NCORES)
    ]
    return in_maps, s_f


def _prep_in_maps(x, f0, f1, f2, f3):
    return _prep(x, f0, f1, f2, f3)[0]


def kernel(x, f0, f1, f2, f3):
    from concourse import bass_utils

    nc = _get_nc()
    in_maps, s_f = _prep(x, f0, f1, f2, f3)
    res = bass_utils.run_bass_kernel_spmd(
        nc, in_maps, core_ids=list(range(NCORES)))
    # shards: [BL, 2, 128, HP*WP] int8; (ft, p) merges to F contiguously.
    raw = np.stack([np.asarray(r["out"]) for r in res.results])
    out = raw.astype(np.float32) * s_f.reshape(1, 1, 2, 128, 1)
    return np.ascontiguousarray(out.reshape(B, F, HP, WP))
